# revision 7
# baseline (speedup 1.0000x reference)
"""GAT (2-layer + classifier) Trainium2 Bass kernel, 8-core SPMD.

Sharding: destination nodes (and their incoming edges) are sharded across 8
cores; projected node features are replicated via AllGather; per-node softmax
numerator and denominator are accumulated per dst tile.

v3 design:
- Nodes within each core are permuted by in-degree so each 128-node dst tile
  has near-uniform degree.  Edge slots are [dst-partition, chunk] with chunk
  count = tile max degree (2.9% padding) -- the segment-sum matmul then uses a
  constant identity lhsT (no per-chunk one-hot build, no a_dst scatter tricks).
- a_dst per edge == a_dst of the partition's own node (broadcast, free).
- One indirect_dma_start (hardware DGE, int32 offsets, no int16 windows) per
  tile gathers source rows; descriptor generation is off the Pool engine.
- Attention dot products folded into augmented weight matrices host-side.
- Everything bf16; gathered h blocks optionally fp8e4m3 (GAT_FP8=1).
- Scale pass (h * softmax weight) split across DVE and Pool engines.
"""

import os
import sys

import numpy as np

sys.path.insert(0, "/opt/trn_rl_repo")

# ---------------- problem constants (hardcoded, from the GAT spec) ---------
N_NODES = 50000
N_EDGES = 800000
IN_DIM = 256
HID = 128
HEADS = 3
N_CLASSES = 40
HC = HEADS * HID  # 384
NEG_SLOPE = 0.2

NCORES = 8
NPC = N_NODES // NCORES  # 6250 nodes per core
TILE = 128
NTILES = (NPC + TILE - 1) // TILE  # 49 (last tile has 106 rows)
ROWE = 392  # row elems: [h(384) | asrc(3) | pad(5)]; fp8: bytes w/ asrc bf16
AUGC = 390  # augmented weight cols: h(384) + asrc(3) + adst(3)
FP8 = os.environ.get("GAT_FP8", "1") == "1"
MASKVAL = -300.0
DVE_FRAC = float(os.environ.get("GAT_DVE_FRAC", 0.62))

_CACHE = {}


def _bf16(a):
    import ml_dtypes

    return np.asarray(a, np.float32).astype(ml_dtypes.bfloat16)


# =========================================================================
# Host-side preprocessing: degree sort, slot assignment, gather offsets
# =========================================================================
def _preprocess(edge_index):
    import ml_dtypes

    src = np.asarray(edge_index[0], dtype=np.int64)
    dst = np.asarray(edge_index[1], dtype=np.int64)
    loops = np.arange(N_NODES, dtype=np.int64)
    src = np.concatenate([src, loops])
    dst = np.concatenate([dst, loops])

    deg = np.bincount(dst, minlength=N_NODES)

    # per-core degree-sorted permutation: perm[slot] = orig local node
    perm = np.zeros((NCORES, NPC), np.int64)
    pos = np.zeros(N_NODES, np.int64)  # global node -> slot within its core
    for r in range(NCORES):
        dg = deg[r * NPC : (r + 1) * NPC]
        p = np.argsort(-dg, kind="stable")
        perm[r] = p
        pos[r * NPC + p] = np.arange(NPC)

    # global row of node n in the AllGather table
    growof = (np.arange(N_NODES) // NPC) * NPC + pos

    # chunk count per tile: max (over cores) of max degree in the sorted tile
    c_t = np.zeros(NTILES, np.int64)
    for r in range(NCORES):
        sd = deg[r * NPC : (r + 1) * NPC][perm[r]]
        for t in range(NTILES):
            c_t[t] = max(c_t[t], sd[TILE * t : TILE * (t + 1)].max())
    oc = np.concatenate([[0], np.cumsum(c_t)])
    CTOT = int(oc[-1])

    IDXI = np.zeros((NCORES, 128, CTOT), np.int32)
    MASKN = np.full((NCORES, 128, CTOT), MASKVAL, ml_dtypes.bfloat16)

    dcore = dst // NPC
    for r in range(NCORES):
        m = dcore == r
        s_r = src[m]
        p_r = pos[dst[m]]  # slot position of dst within core
        order = np.argsort(p_r, kind="stable")
        p_s = p_r[order]
        s_s = growof[s_r[order]]  # gather row of the source node
        starts = np.searchsorted(p_s, np.arange(NPC))
        rank = np.arange(len(p_s)) - starts[p_s]
        t_s = p_s // TILE
        j_s = p_s % TILE
        cols = oc[t_s] + rank
        IDXI[r, j_s, cols] = s_s.astype(np.int32)
        MASKN[r, j_s, cols] = 0.0

    sched = dict(c_t=[int(v) for v in c_t], oc=[int(v) for v in oc], CTOT=CTOT)
    return sched, IDXI, MASKN, perm


def _augment(W, att_src, att_dst):
    """[K, 384] -> [K, 390] with per-head att_src/att_dst projections."""
    W = np.asarray(W, np.float32)
    cols = [W]
    for att in (att_src, att_dst):
        a = np.zeros((W.shape[0], HEADS), np.float32)
        for h in range(HEADS):
            a[:, h] = W[:, HID * h : HID * (h + 1)] @ np.asarray(att[h], np.float32)
        cols.append(a)
    return np.concatenate(cols, axis=1)  # [K, 390]


def _shared_inputs(W1, att_src1, att_dst1, b1, W2, att_src2, att_dst2, b2, outW, outb):
    f = np.float32
    return {
        "W1A": _bf16(_augment(W1, att_src1, att_dst1)),  # [256, 390]
        "W2A": _bf16(_augment(W2, att_src2, att_dst2)),  # [384, 390]
        "OUTW": _bf16(outW),  # [384, 40]
        "B1R": _bf16(np.tile(np.asarray(b1, f)[None, :], (128, 1))),
        "B2R": _bf16(np.tile(np.asarray(b2, f)[None, :], (128, 1))),
        "OUTBR": np.tile(np.asarray(outb, f)[None, :], (128, 1)),
        "IDENT": _bf16(np.eye(128, dtype=f)),
    }


# =========================================================================
# Bass program
# =========================================================================
def _build_program(sched):
    from contextlib import ExitStack

    import concourse.bass as bass
    import concourse.mybir as mybir
    import concourse.tile as tile
    from concourse import bacc

    f32 = mybir.dt.float32
    bf16 = mybir.dt.bfloat16
    fp8 = mybir.dt.float8e4
    i32 = mybir.dt.int32
    gdt = fp8 if FP8 else bf16
    AF = mybir.ActivationFunctionType
    OP = mybir.AluOpType
    AP = bass.AP

    c_t, oc, CTOT = sched["c_t"], sched["oc"], sched["CTOT"]

    nc = bacc.Bacc(
        "TRN2",
        target_bir_lowering=False,
        debug=False,
        enable_asserts=False,
        num_devices=NCORES,
        num_swdge_queues=2,
        dynamic_dma_scratch_size=int(os.environ.get("GAT_DMA_SCRATCH", 16384)),
    )

    # ---- I/O ----
    XTT = nc.dram_tensor("XTT", [NTILES * IN_DIM, TILE], bf16, kind="ExternalInput")
    IDXI = nc.dram_tensor("IDXI", [128, CTOT], i32, kind="ExternalInput")
    MASKN = nc.dram_tensor("MASKN", [128, CTOT], bf16, kind="ExternalInput")
    W1A = nc.dram_tensor("W1A", [IN_DIM, AUGC], bf16, kind="ExternalInput")
    W2A = nc.dram_tensor("W2A", [HC, AUGC], bf16, kind="ExternalInput")
    OUTW = nc.dram_tensor("OUTW", [HC, N_CLASSES], bf16, kind="ExternalInput")
    B1R = nc.dram_tensor("B1R", [128, HC], bf16, kind="ExternalInput")
    B2R = nc.dram_tensor("B2R", [128, HC], bf16, kind="ExternalInput")
    OUTBR = nc.dram_tensor("OUTBR", [128, N_CLASSES], f32, kind="ExternalInput")
    IDENT = nc.dram_tensor("IDENT", [128, 128], bf16, kind="ExternalInput")
    OUT = nc.dram_tensor("OUT", [NPC, N_CLASSES], f32, kind="ExternalOutput")

    def mid_bcast(ap2d, count):
        # [128, A] -> [128, count, A] with the middle dim broadcast (step 0)
        return AP(ap2d.tensor, ap2d.offset, [ap2d.ap[0], [0, count], ap2d.ap[1]])

    def col_bcast3(ap2d, count):
        # [128, C] -> [128, C, count] (trailing broadcast of each column)
        return AP(ap2d.tensor, ap2d.offset, [ap2d.ap[0], ap2d.ap[1], [0, count]])

    with tile.TileContext(nc) as tc, ExitStack() as ctx:
        cpool = ctx.enter_context(tc.tile_pool(name="cpool", bufs=1))
        dram = ctx.enter_context(tc.tile_pool(name="dram", bufs=1, space="DRAM"))
        gpool = ctx.enter_context(tc.tile_pool(name="gpool", bufs=2))
        wpool = ctx.enter_context(tc.tile_pool(name="wpool", bufs=2))
        ppool = ctx.enter_context(tc.tile_pool(name="ppool", bufs=2, space="PSUM"))
        apool = ctx.enter_context(tc.tile_pool(name="apool", bufs=3, space="PSUM"))

        # resident constants
        idxi_sb = cpool.tile_from(IDXI.ap())
        maskn_sb = cpool.tile_from(MASKN.ap())
        b1r_sb = cpool.tile_from(B1R.ap())
        b2r_sb = cpool.tile_from(B2R.ap())
        outbr_sb = cpool.tile_from(OUTBR.ap())
        ident_sb = cpool.tile_from(IDENT.ap())
        w1_sb = [
            cpool.tile_from(W1A.ap()[128 * k : 128 * (k + 1), :], name=f"w1_{k}")
            for k in range(2)
        ]
        w2_sb = [
            cpool.tile_from(W2A.ap()[128 * k : 128 * (k + 1), :], name=f"w2_{k}")
            for k in range(3)
        ]
        outw_sb = [
            cpool.tile_from(OUTW.ap()[128 * k : 128 * (k + 1), :], name=f"outw_{k}")
            for k in range(3)
        ]
        adst = cpool.tile([128, NTILES * HEADS], bf16)  # per-layer a_dst per tile

        agin1 = dram.tile([NPC, ROWE], gdt)
        hext1 = dram.tile([N_NODES, ROWE], gdt, addr_space="Shared")
        agin2 = dram.tile([NPC, ROWE], gdt)
        hext2 = dram.tile([N_NODES, ROWE], gdt, addr_space="Shared")

        def rows_of(t):
            return min(TILE, NPC - t * TILE)

        def pack_row(t, src_psum):
            """psum [128, 390] = [h(384)|asrc(3)|adst(3)] -> row [128, ROWE]."""
            row = wpool.tile([128, ROWE], gdt, tag="row")
            nc.vector.tensor_copy(row[:, 0:HC], src_psum[:, 0:HC])
            if FP8:
                rb = row[:].bitcast(bf16)  # [128, 196]
                nc.vector.tensor_copy(rb[:, 192:195], src_psum[:, HC : HC + 3])
                nc.gpsimd.memset(row[:, 390:392], 0.0)
            else:
                nc.vector.tensor_copy(row[:, HC : HC + 3], src_psum[:, HC : HC + 3])
                nc.gpsimd.memset(row[:, HC + 3 : ROWE], 0.0)
            nc.vector.tensor_copy(
                adst[:, HEADS * t : HEADS * (t + 1)], src_psum[:, HC + 3 : HC + 6]
            )
            return row

        # ---------------- Phase 1: h1 = x @ W1A, pack rows ------------------
        for t in range(NTILES):
            h1_ps = apool.tile([128, AUGC], f32, tag="acc")
            for k in range(2):
                xk = wpool.tile([128, 128], bf16, tag="xk")
                nc.sync.dma_start(
                    out=xk[:],
                    in_=XTT.ap()[IN_DIM * t + 128 * k : IN_DIM * t + 128 * (k + 1), :],
                )
                nc.tensor.matmul(
                    h1_ps[:], lhsT=xk[:], rhs=w1_sb[k][:], start=(k == 0), stop=(k == 1)
                )
            row = pack_row(t, h1_ps)
            r = rows_of(t)
            nc.sync.dma_start(out=agin1[TILE * t : TILE * t + r, :], in_=row[:r, :])

        nc.gpsimd.collective_compute(
            "AllGather",
            mybir.AluOpType.bypass,
            replica_groups=[list(range(NCORES))],
            ins=[agin1[:]],
            outs=[hext1[:]],
        )

        # ---------------- Edge pass (shared for both layers) ----------------
        def h4(tile_ap, c0, cnt, rowlen):
            # 4D head-block view [128, cnt, 3, 128] of chunks c0..c0+cnt
            a = tile_ap
            return AP(
                a.tensor, a.offset + c0 * rowlen,
                [a.ap[0], [rowlen, cnt], [HID, HEADS], [1, HID]],
            )

        def ex4(ex_ap, c0, cnt):
            a = ex_ap
            return AP(
                a.tensor, a.offset + c0 * HEADS,
                [a.ap[0], [HEADS, cnt], [1, HEADS], [0, HID]],
            )

        def edge_pass(t, hext):
            c = c_t[t]
            G = gpool.tile([128, c, ROWE], gdt, tag="G")
            for ci in range(c):
                nc.gpsimd.indirect_dma_start(
                    out=G[:, ci, :],
                    out_offset=None,
                    in_=hext[:],
                    in_offset=bass.IndirectOffsetOnAxis(
                        ap=idxi_sb[:, oc[t] + ci : oc[t] + ci + 1], axis=0
                    ),
                )
            # alpha = asrc[src] + adst[dst] + mask ; leaky ; exp
            if FP8:
                gb = G[:].bitcast(bf16)  # [128, c, 196]
                asrcv = gb[:, :, 192:195]
            else:
                asrcv = G[:, :, HC : HC + 3]
            alpha = wpool.tile([128, c, HEADS], bf16, tag="alpha")
            nc.vector.tensor_tensor(
                out=alpha[:], in0=asrcv,
                in1=mid_bcast(adst[:, HEADS * t : HEADS * (t + 1)], c), op=OP.add,
            )
            nc.vector.tensor_tensor(
                out=alpha[:], in0=alpha[:],
                in1=col_bcast3(maskn_sb[:, oc[t] : oc[t] + c], HEADS), op=OP.add,
            )
            nc.vector.scalar_tensor_tensor(
                out=alpha[:], in0=alpha[:], scalar=NEG_SLOPE, in1=alpha[:],
                op0=OP.mult, op1=OP.max,
            )
            ex = wpool.tile([128, c, HEADS], bf16, tag="ex")
            nc.scalar.activation(ex[:], alpha[:], AF.Exp)
            # Gs = G.h * ex   (split across DVE and Pool)
            Gs = gpool.tile([128, c, HC], bf16, tag="Gs")
            cd = max(0, min(c, int(round(c * DVE_FRAC))))
            if cd > 0:
                nc.vector.tensor_tensor(
                    out=h4(Gs[:], 0, cd, HC), in0=h4(G[:], 0, cd, ROWE),
                    in1=ex4(ex[:], 0, cd), op=OP.mult,
                )
            if c - cd > 0:
                nc.gpsimd.tensor_tensor(
                    out=h4(Gs[:], cd, c - cd, HC), in0=h4(G[:], cd, c - cd, ROWE),
                    in1=ex4(ex[:], cd, c - cd), op=OP.mult,
                )
            # denominators: den[j, h] = sum_ci ex[j, ci, h]
            den = wpool.tile([128, HEADS], f32, tag="den")
            exT = AP(ex[:].tensor, ex[:].offset, [ex[:].ap[0], [1, HEADS], [HEADS, c]])
            nc.vector.tensor_reduce(
                out=den[:], in_=exT, axis=mybir.AxisListType.X, op=OP.add
            )
            # segment-sum via identity-lhsT accumulating matmuls
            out_ps = apool.tile([128, HC], f32, tag="acc")
            for ci in range(c):
                nc.tensor.matmul(
                    out_ps[:],
                    lhsT=ident_sb[:],
                    rhs=Gs[:, ci, :],
                    start=(ci == 0),
                    stop=(ci == c - 1),
                )
            return out_ps, den

        def normalize(out_ps, den, brep_sb):
            """h = relu(out/den + bias)  -> [128, 384] bf16 sbuf tile"""
            tmp3 = wpool.tile([128, HEADS], f32, tag="tmp3")
            nc.vector.tensor_scalar_add(tmp3[:], den[:], 1e-16)
            r3 = wpool.tile([128, HEADS], f32, tag="r3")
            nc.vector.reciprocal(r3[:], tmp3[:])
            h2 = wpool.tile([128, HC], bf16, tag="h2")
            for h in range(HEADS):
                nc.vector.scalar_tensor_tensor(
                    out=h2[:, HID * h : HID * (h + 1)],
                    in0=out_ps[:, HID * h : HID * (h + 1)],
                    scalar=r3[:, h : h + 1],
                    in1=brep_sb[:, HID * h : HID * (h + 1)],
                    op0=OP.mult,
                    op1=OP.add,
                )
            nc.vector.tensor_scalar_max(h2[:], h2[:], 0.0)
            return h2

        # ---------------- Phase 2: edge pass L1 + entry L2 ------------------
        limit = int(os.environ.get("GAT_LIMIT_TILES", NTILES))
        for t in range(min(NTILES, limit)):
            out_ps, den = edge_pass(t, hext1)
            h2 = normalize(out_ps, den, b1r_sb)
            h3_ps = apool.tile([128, AUGC], f32, tag="acc")
            for k in range(3):
                tp = ppool.tile([128, 128], bf16, tag="sq")
                nc.tensor.transpose(tp[:], h2[:, 128 * k : 128 * (k + 1)], ident_sb[:])
                h2T = wpool.tile([128, 128], bf16, tag="h2T", bufs=3)
                nc.scalar.activation(h2T[:], tp[:], AF.Copy)
                nc.tensor.matmul(
                    h3_ps[:], lhsT=h2T[:], rhs=w2_sb[k][:], start=(k == 0), stop=(k == 2)
                )
            row = pack_row(t, h3_ps)
            r = rows_of(t)
            nc.sync.dma_start(out=agin2[TILE * t : TILE * t + r, :], in_=row[:r, :])

        nc.gpsimd.collective_compute(
            "AllGather",
            mybir.AluOpType.bypass,
            replica_groups=[list(range(NCORES))],
            ins=[agin2[:]],
            outs=[hext2[:]],
        )

        # ---------------- Phase 3: edge pass L2 + classifier ----------------
        for t in range(min(NTILES, limit)):
            out_ps, den = edge_pass(t, hext2)
            h3 = normalize(out_ps, den, b2r_sb)
            cls_ps = ppool.tile([128, N_CLASSES], f32, tag="cls")
            for k in range(3):
                tp = ppool.tile([128, 128], bf16, tag="sq")
                nc.tensor.transpose(tp[:], h3[:, 128 * k : 128 * (k + 1)], ident_sb[:])
                h3T = wpool.tile([128, 128], bf16, tag="h2T", bufs=3)
                nc.scalar.activation(h3T[:], tp[:], AF.Copy)
                nc.tensor.matmul(
                    cls_ps[:], lhsT=h3T[:], rhs=outw_sb[k][:], start=(k == 0), stop=(k == 2)
                )
            outt = wpool.tile([128, N_CLASSES], f32, tag="outt")
            nc.vector.tensor_tensor(out=outt[:], in0=cls_ps[:], in1=outbr_sb[:], op=OP.add)
            r = rows_of(t)
            nc.sync.dma_start(out=OUT.ap()[TILE * t : TILE * t + r, :], in_=outt[:r, :])

    nc.compile()
    return nc


# =========================================================================
# entry point
# =========================================================================
def _prepare(inputs):
    """Build (cached) program + per-core input maps from FULL inputs."""
    import ml_dtypes

    x = np.asarray(inputs["x"], np.float32)
    edge_index = np.asarray(inputs["edge_index"])

    key = "prog"
    if key not in _CACHE:
        sched, IDXI, MASKN, perm = _preprocess(edge_index)
        nc = _build_program(sched)
        _CACHE[key] = (sched, IDXI, MASKN, perm, nc)
    sched, IDXI, MASKN, perm, nc = _CACHE[key]

    shared = _shared_inputs(
        inputs["W1"], inputs["att_src1"], inputs["att_dst1"], inputs["b1"],
        inputs["W2"], inputs["att_src2"], inputs["att_dst2"], inputs["b2"],
        inputs["outW"], inputs["outb"],
    )

    in_maps = []
    for r in range(NCORES):
        xs = x[r * NPC : (r + 1) * NPC][perm[r]]  # [NPC, 256] degree-sorted
        xtt = np.zeros((NTILES * IN_DIM, TILE), ml_dtypes.bfloat16)
        for t in range(NTILES):
            rt = min(TILE, NPC - t * TILE)
            xtt[IN_DIM * t : IN_DIM * (t + 1), :rt] = (
                xs[TILE * t : TILE * t + rt].T.astype(ml_dtypes.bfloat16)
            )
        m = dict(shared)
        m["XTT"] = xtt
        m["IDXI"] = IDXI[r]
        m["MASKN"] = MASKN[r]
        in_maps.append(m)
    return nc, in_maps


def kernel(**inputs):
    nc, in_maps = _prepare(inputs)
    _, _, _, perm, _ = _CACHE["prog"]

    from concourse.bass_utils import run_bass_kernel_spmd

    res = run_bass_kernel_spmd(nc, in_maps, core_ids=list(range(NCORES)))
    out = np.empty((N_NODES, N_CLASSES), np.float32)
    for r in range(NCORES):
        out[r * NPC + perm[r]] = res.results[r]["OUT"]
    return out


if __name__ == "__main__":
    sys.path.insert(0, os.path.dirname(os.path.abspath(__file__)))
    import reference

    inp = {k: np.asarray(v) for k, v in reference.setup_inputs().items()}
    got = kernel(**inp)
    exp = np.asarray(reference.reference(**reference.setup_inputs()))
    err = np.abs(got - exp).max() / (np.abs(exp).max() + 1e-12)
    print("rel err:", err)


# revision 9
# speedup vs baseline: 1.4978x; 1.4978x over previous
"""GAT (2-layer + classifier) Trainium2 Bass kernel, 8-core SPMD.

Sharding: destination nodes (and their incoming edges, sorted by dst) are
sharded across 8 cores; projected node features are replicated via AllGather;
per-node softmax numerator and denominator are accumulated with the weighted
message matmul (ones-column trick), so no cross-core reduction is needed.

v4 design:
- Gathered table rows are fp8e4m3 h-blocks (+bf16 a_src) in 512B rows: halves
  HBM gather traffic and Pool-engine descriptor work vs f32.
- The one-hot scatter matrices (iseq: edge-slot -> dst col; iseqT: transpose,
  for the per-edge a_dst pick) are STATIC given the edge list, so they are
  precomputed host-side and streamed from DRAM instead of built on-chip
  (v2 spent 1.1ms of DVE building them; DMA has headroom, DVE does not).
- Attention dot products are folded into augmented weight matrices host-side.
- The per-edge softmax scale of gathered rows runs split across DVE (heads
  0-1) and the scalar engine (head 2, per-partition activation scale).
"""

import os
import sys

import numpy as np

sys.path.insert(0, "/opt/trn_rl_repo")

# ---------------- problem constants (hardcoded, from the GAT spec) ---------
N_NODES = 50000
N_EDGES = 800000
IN_DIM = 256
HID = 128
HEADS = 3
N_CLASSES = 40
HC = HEADS * HID  # 384
NEG_SLOPE = 0.2

NCORES = 8
NPC = N_NODES // NCORES  # 6250 nodes per core
TILE = 128
NTILES = (NPC + TILE - 1) // TILE  # 49 (last tile has 106 rows)
MMN = 387  # matmul N: 3*(128+1), interleaved [h|1] blocks
AUGC = 390  # augmented weight cols: h(384) + asrc(3) + adst(3)
WIN = 32768  # int16 index window
FP8 = os.environ.get("GAT_FP8", "1") == "1"
ROWE = 512 if FP8 else 448  # gather row elems; bytes must be %256
ASRCB = 194 if FP8 else None  # bf16 idx of asrc within bitcast row (fp8)
ACT_HEAD = os.environ.get("GAT_ACT_HEAD", "1") == "1"  # head 2 scale on Act

_CACHE = {}


def _round_up(x, m):
    return (x + m - 1) // m * m


def _bf16(a):
    import ml_dtypes

    return np.asarray(a, np.float32).astype(ml_dtypes.bfloat16)


# =========================================================================
# Host-side preprocessing: edge sort / shard / pad, idx + static one-hots
# =========================================================================
def _preprocess(edge_index):
    import ml_dtypes

    src = np.asarray(edge_index[0], dtype=np.int64)
    dst = np.asarray(edge_index[1], dtype=np.int64)
    loops = np.arange(N_NODES, dtype=np.int64)
    src = np.concatenate([src, loops])
    dst = np.concatenate([dst, loops])

    core = dst // NPC
    rel = dst - core * NPC
    tile_i = rel // TILE
    win = (src >= WIN).astype(np.int64)
    key = (core * NTILES + tile_i) * 2 + win
    order = np.argsort(key, kind="stable")
    src = src[order]
    rel = rel[order]

    counts = np.bincount(key[order], minlength=NCORES * NTILES * 2)
    starts = np.zeros(NCORES * NTILES * 2 + 1, np.int64)
    np.cumsum(counts, out=starts[1:])
    cnt = counts.reshape(NCORES, NTILES, 2)

    p0 = _round_up(cnt[:, :, 0].max(axis=0), 128)  # [NTILES] padded win0 len
    p1 = _round_up(cnt[:, :, 1].max(axis=0), 128)
    c_t = (p0 + p1) // 128  # chunks per tile
    T0 = int(p0.sum())
    T1 = int(p1.sum())
    CTOT = int(c_t.sum())
    o0 = np.concatenate([[0], np.cumsum(p0)])  # idx elem offsets
    o1 = np.concatenate([[0], np.cumsum(p1)])
    oc = np.concatenate([[0], np.cumsum(c_t)])  # chunk offsets

    IDX0 = np.zeros((NCORES, 128, T0 // 16), np.int16)
    IDX1 = np.zeros((NCORES, 128, T1 // 16), np.int16)
    ISEQ = np.zeros((NCORES, 128, CTOT * 128), ml_dtypes.bfloat16)
    ISEQT = np.zeros((NCORES, 128, CTOT * 128), ml_dtypes.bfloat16)
    d128 = np.arange(128)

    def wrap16(a):  # idx i -> [i%16, i//16], replicated to 128 partitions
        w = a.reshape(-1, 16).T
        return np.tile(w, (8, 1))

    for r in range(NCORES):
        for t in range(NTILES):
            c = c_t[t]
            drel_tile = np.full(128 * c, -1.0, np.float32)
            for w, (P, O, IDX, base, off_in) in enumerate(
                ((p0, o0, IDX0, 0, 0), (p1, o1, IDX1, WIN, p0[t]))
            ):
                k = (r * NTILES + t) * 2 + w
                s, e = starts[k], starts[k + 1]
                n = e - s
                idxs = np.zeros(P[t], np.int16)
                if n > 0:
                    idxs[:n] = (src[s:e] - base).astype(np.int16)
                    idxs[n:] = idxs[n - 1]
                    drel_tile[off_in : off_in + n] = (rel[s:e] - t * TILE).astype(
                        np.float32
                    )
                if P[t] > 0:
                    IDX[r, :, O[t] // 16 : (O[t] + P[t]) // 16] = wrap16(idxs)
            dt = drel_tile.reshape(c, 128)  # [chunk, edge-slot]
            oh = dt[:, :, None] == d128[None, None, :]  # [c, e, d]
            sl = slice(oc[t] * 128, (oc[t] + c) * 128)
            ISEQ[r, :, sl] = (
                oh.transpose(1, 0, 2).reshape(128, c * 128).astype(ml_dtypes.bfloat16)
            )
            ISEQT[r, :, sl] = (
                oh.transpose(2, 0, 1).reshape(128, c * 128).astype(ml_dtypes.bfloat16)
            )

    sched = dict(
        p0=[int(v) for v in p0],
        p1=[int(v) for v in p1],
        c_t=[int(v) for v in c_t],
        o0=[int(v) for v in o0],
        o1=[int(v) for v in o1],
        oc=[int(v) for v in oc],
        T0=T0,
        T1=T1,
        CTOT=CTOT,
    )
    return sched, IDX0, IDX1, ISEQ, ISEQT


def _augment(W, att_src, att_dst):
    """[K, 384] -> [K, 390] with per-head att_src/att_dst projections."""
    W = np.asarray(W, np.float32)
    cols = [W]
    for att in (att_src, att_dst):
        a = np.zeros((W.shape[0], HEADS), np.float32)
        for h in range(HEADS):
            a[:, h] = W[:, HID * h : HID * (h + 1)] @ np.asarray(att[h], np.float32)
        cols.append(a)
    return np.concatenate(cols, axis=1)  # [K, 390]


def _shared_inputs(W1, att_src1, att_dst1, b1, W2, att_src2, att_dst2, b2, outW, outb):
    f = np.float32
    return {
        "W1A": _bf16(_augment(W1, att_src1, att_dst1)),  # [256, 390]
        "W2A": _bf16(_augment(W2, att_src2, att_dst2)),  # [384, 390]
        "OUTW": _bf16(outW),  # [384, 40]
        "B1R": _bf16(np.tile(np.asarray(b1, f)[None, :], (128, 1))),
        "B2R": _bf16(np.tile(np.asarray(b2, f)[None, :], (128, 1))),
        "OUTBR": np.tile(np.asarray(outb, f)[None, :], (128, 1)),
        "IDENT": _bf16(np.eye(128, dtype=f)),
    }


# =========================================================================
# Bass program
# =========================================================================
def _build_program(sched):
    from contextlib import ExitStack

    import concourse.bass as bass
    import concourse.mybir as mybir
    import concourse.tile as tile
    from concourse import bacc

    f32 = mybir.dt.float32
    bf16 = mybir.dt.bfloat16
    fp8 = mybir.dt.float8e4
    i16 = mybir.dt.int16
    gdt = fp8 if FP8 else f32
    AF = mybir.ActivationFunctionType
    OP = mybir.AluOpType
    AP = bass.AP

    p0, p1, c_t = sched["p0"], sched["p1"], sched["c_t"]
    o0, o1, oc = sched["o0"], sched["o1"], sched["oc"]
    T0, T1, CTOT = sched["T0"], sched["T1"], sched["CTOT"]

    nc = bacc.Bacc(
        "TRN2",
        target_bir_lowering=False,
        debug=False,
        enable_asserts=False,
        num_devices=NCORES,
        num_swdge_queues=2,
        dynamic_dma_scratch_size=int(os.environ.get("GAT_DMA_SCRATCH", 16384)),
    )

    # ---- I/O ----
    XTT = nc.dram_tensor("XTT", [NTILES * IN_DIM, TILE], bf16, kind="ExternalInput")
    IDX0 = nc.dram_tensor("IDX0", [128, T0 // 16], i16, kind="ExternalInput")
    IDX1 = nc.dram_tensor("IDX1", [128, T1 // 16], i16, kind="ExternalInput")
    ISEQ = nc.dram_tensor("ISEQ", [128, CTOT * 128], bf16, kind="ExternalInput")
    ISEQT = nc.dram_tensor("ISEQT", [128, CTOT * 128], bf16, kind="ExternalInput")
    W1A = nc.dram_tensor("W1A", [IN_DIM, AUGC], bf16, kind="ExternalInput")
    W2A = nc.dram_tensor("W2A", [HC, AUGC], bf16, kind="ExternalInput")
    OUTW = nc.dram_tensor("OUTW", [HC, N_CLASSES], bf16, kind="ExternalInput")
    B1R = nc.dram_tensor("B1R", [128, HC], bf16, kind="ExternalInput")
    B2R = nc.dram_tensor("B2R", [128, HC], bf16, kind="ExternalInput")
    OUTBR = nc.dram_tensor("OUTBR", [128, N_CLASSES], f32, kind="ExternalInput")
    IDENT = nc.dram_tensor("IDENT", [128, 128], bf16, kind="ExternalInput")
    OUT = nc.dram_tensor("OUT", [NPC, N_CLASSES], f32, kind="ExternalOutput")

    def strided3(ap2d, start, step, count):
        # [128, N] -> [128, count] picking cols start, start+step, ...
        base = ap2d[:, start : start + 1]
        return AP(base.tensor, base.offset, [base.ap[0], [step, count]])

    def seg_view(ap2d, nseg, seglen, stride):
        # [128, N] -> [128, nseg, seglen] with segment stride `stride`
        return AP(ap2d.tensor, ap2d.offset, [ap2d.ap[0], [stride, nseg], [1, seglen]])

    with tile.TileContext(nc) as tc, ExitStack() as ctx:
        cpool = ctx.enter_context(tc.tile_pool(name="cpool", bufs=1))
        dram = ctx.enter_context(tc.tile_pool(name="dram", bufs=1, space="DRAM"))
        gpool = ctx.enter_context(tc.tile_pool(name="gpool", bufs=2))
        wpool = ctx.enter_context(tc.tile_pool(name="wpool", bufs=2))
        ppool = ctx.enter_context(tc.tile_pool(name="ppool", bufs=2, space="PSUM"))
        apool = ctx.enter_context(tc.tile_pool(name="apool", bufs=3, space="PSUM"))

        # resident constants
        idx0_sb = cpool.tile_from(IDX0.ap())
        idx1_sb = cpool.tile_from(IDX1.ap())
        b1r_sb = cpool.tile_from(B1R.ap())
        b2r_sb = cpool.tile_from(B2R.ap())
        outbr_sb = cpool.tile_from(OUTBR.ap())
        ident_sb = cpool.tile_from(IDENT.ap())
        w1_sb = [
            cpool.tile_from(W1A.ap()[128 * k : 128 * (k + 1), :], name=f"w1_{k}")
            for k in range(2)
        ]
        w2_sb = [
            cpool.tile_from(W2A.ap()[128 * k : 128 * (k + 1), :], name=f"w2_{k}")
            for k in range(3)
        ]
        outw_sb = [
            cpool.tile_from(OUTW.ap()[128 * k : 128 * (k + 1), :], name=f"outw_{k}")
            for k in range(3)
        ]
        adst = cpool.tile([128, NTILES * HEADS], bf16)  # per-layer a_dst per tile

        agin1 = dram.tile([NPC, ROWE], gdt)
        hext1 = dram.tile([N_NODES, ROWE], gdt, addr_space="Shared")
        agin2 = dram.tile([NPC, ROWE], gdt)
        hext2 = dram.tile([N_NODES, ROWE], gdt, addr_space="Shared")

        def rows_of(t):
            return min(TILE, NPC - t * TILE)

        def pack_row(t, src_psum):
            """psum [128, 390] = [h(384)|asrc(3)|adst(3)] -> table row tile.

            Row: fp8 [h0|1|h1|1|h2|1 (387)|pad|asrc bf16@byte388|pad] (512B)
                 f32 [h0|1|h1|1|h2|1 (387)|asrc(3)|pad] (448 elems)
            """
            row = wpool.tile([128, ROWE], gdt, tag="row")
            nc.gpsimd.memset(row[:, MMN:ROWE], 0.0)
            nc.vector.tensor_copy(
                seg_view(row, HEADS, HID, HID + 1), seg_view(src_psum, HEADS, HID, HID)
            )
            nc.vector.memset(strided3(row, HID, HID + 1, HEADS), 1.0)
            if FP8:
                rb = row[:].bitcast(bf16)  # [128, 256]
                nc.vector.tensor_copy(
                    rb[:, ASRCB : ASRCB + 3], src_psum[:, HC : HC + 3]
                )
            else:
                nc.vector.tensor_copy(row[:, MMN : MMN + 3], src_psum[:, HC : HC + 3])
            nc.vector.tensor_copy(
                adst[:, HEADS * t : HEADS * (t + 1)], src_psum[:, HC + 3 : HC + 6]
            )
            return row

        # ---------------- Phase 1: h1 = x @ W1A, pack rows ------------------
        for t in range(NTILES):
            h1_ps = apool.tile([128, AUGC], f32, tag="acc")
            for k in range(2):
                xk = wpool.tile([128, 128], bf16, tag="xk")
                nc.sync.dma_start(
                    out=xk[:],
                    in_=XTT.ap()[IN_DIM * t + 128 * k : IN_DIM * t + 128 * (k + 1), :],
                )
                nc.tensor.matmul(
                    h1_ps[:], lhsT=xk[:], rhs=w1_sb[k][:], start=(k == 0), stop=(k == 1)
                )
            row = pack_row(t, h1_ps)
            r = rows_of(t)
            nc.sync.dma_start(out=agin1[TILE * t : TILE * t + r, :], in_=row[:r, :])

        nc.gpsimd.collective_compute(
            "AllGather",
            mybir.AluOpType.bypass,
            replica_groups=[list(range(NCORES))],
            ins=[agin1[:]],
            outs=[hext1[:]],
        )

        # ---------------- Edge pass (shared for both layers) ----------------
        def edge_pass(t, hext):
            c = c_t[t]
            q0 = p0[t] // 128
            G = gpool.tile([128, c, ROWE], gdt, tag="G")
            if p0[t] > 0:
                nc.gpsimd.dma_gather(
                    out_ap=G[:, :q0, :],
                    in_ap=hext[0:WIN, :],
                    idxs_ap=idx0_sb[:, o0[t] // 16 : (o0[t] + p0[t]) // 16],
                    num_idxs=p0[t],
                    num_idxs_reg=p0[t],
                    elem_size=ROWE,
                    queue_num=0,
                    single_packet=False,
                )
            if p1[t] > 0:
                nc.gpsimd.dma_gather(
                    out_ap=G[:, q0:c, :],
                    in_ap=hext[WIN:N_NODES, :],
                    idxs_ap=idx1_sb[:, o1[t] // 16 : (o1[t] + p1[t]) // 16],
                    num_idxs=p1[t],
                    num_idxs_reg=p1[t],
                    elem_size=ROWE,
                    queue_num=1,
                    single_packet=False,
                )
            # stream the static one-hot blocks for this tile
            iseq = wpool.tile([128, c, 128], bf16, tag="iseq")
            nc.sync.dma_start(
                out=iseq[:], in_=ISEQ.ap()[:, oc[t] * 128 : (oc[t] + c) * 128]
            )
            iseqT = wpool.tile([128, c, 128], bf16, tag="iseqT")
            nc.sync.dma_start(
                out=iseqT[:], in_=ISEQT.ap()[:, oc[t] * 128 : (oc[t] + c) * 128]
            )
            # a_dst per edge: dcol[:, ci, :] = iseqT_ci^T @ adst_t
            dcol_ps = ppool.tile([128, c, HEADS], f32, tag="dcol")
            for ci in range(c):
                nc.tensor.matmul(
                    dcol_ps[:, ci, :],
                    lhsT=iseqT[:, ci, :],
                    rhs=adst[:, HEADS * t : HEADS * (t + 1)],
                    start=True,
                    stop=True,
                )
            # alpha / leaky relu / exp   [128, c, 3] bf16
            if FP8:
                gb = G[:].bitcast(bf16)  # [128, c, 256]
                asrcv = gb[:, :, ASRCB : ASRCB + 3]
            else:
                asrcv = G[:, :, MMN : MMN + 3]
            alpha = wpool.tile([128, c, HEADS], bf16, tag="alpha")
            nc.vector.tensor_tensor(
                out=alpha[:], in0=asrcv, in1=dcol_ps[:], op=OP.add
            )
            nc.vector.scalar_tensor_tensor(
                out=alpha[:], in0=alpha[:], scalar=NEG_SLOPE, in1=alpha[:],
                op0=OP.mult, op1=OP.max,
            )
            ex = wpool.tile([128, c, HEADS], f32, tag="ex")
            nc.scalar.activation(ex[:], alpha[:], AF.Exp)
            # Gs = G[0:387] * ex (per 129-col head block; ones cols -> denom)
            Gs = gpool.tile([128, c, MMN], bf16, tag="Gs")
            nh = 2 if ACT_HEAD else 3
            gt, st, et = G[:], Gs[:], ex[:]
            g4 = AP(gt.tensor, gt.offset, [gt.ap[0], [ROWE, c], [HID + 1, nh], [1, HID + 1]])
            s4 = AP(st.tensor, st.offset, [st.ap[0], [MMN, c], [HID + 1, nh], [1, HID + 1]])
            e4 = AP(et.tensor, et.offset, [et.ap[0], [HEADS, c], [1, nh], [0, HID + 1]])
            nc.vector.tensor_tensor(out=s4, in0=g4, in1=e4, op=OP.mult)
            if ACT_HEAD:
                for ci in range(c):
                    nc.scalar.activation(
                        Gs[:, ci, 2 * (HID + 1) : MMN],
                        G[:, ci, 2 * (HID + 1) : MMN],
                        AF.Copy,
                        scale=ex[:, ci, 2:3],
                    )
            # weighted segment sum (and denominators via the ones columns)
            out_ps = apool.tile([128, MMN], f32, tag="acc")
            for ci in range(c):
                nc.tensor.matmul(
                    out_ps[:],
                    lhsT=iseq[:, ci, :],
                    rhs=Gs[:, ci, :],
                    start=(ci == 0),
                    stop=(ci == c - 1),
                )
            return out_ps

        def normalize(out_ps, brep_sb):
            """h = relu(out/denom + bias)  -> [128, 384] bf16 sbuf tile"""
            tmp3 = wpool.tile([128, HEADS], f32, tag="tmp3")
            nc.vector.tensor_scalar_add(
                tmp3[:], strided3(out_ps, HID, HID + 1, HEADS), 1e-16
            )
            r3 = wpool.tile([128, HEADS], f32, tag="r3")
            nc.vector.reciprocal(r3[:], tmp3[:])
            h2 = wpool.tile([128, HC], bf16, tag="h2")
            for h in range(HEADS):
                nc.vector.scalar_tensor_tensor(
                    out=h2[:, HID * h : HID * (h + 1)],
                    in0=out_ps[:, (HID + 1) * h : (HID + 1) * h + HID],
                    scalar=r3[:, h : h + 1],
                    in1=brep_sb[:, HID * h : HID * (h + 1)],
                    op0=OP.mult,
                    op1=OP.add,
                )
            nc.vector.tensor_scalar_max(h2[:], h2[:], 0.0)
            return h2

        # ---------------- Phase 2: edge pass L1 + entry L2 ------------------
        limit = int(os.environ.get("GAT_LIMIT_TILES", NTILES))
        for t in range(min(NTILES, limit)):
            out_ps = edge_pass(t, hext1)
            h2 = normalize(out_ps, b1r_sb)
            h3_ps = apool.tile([128, AUGC], f32, tag="acc")
            for k in range(3):
                tp = ppool.tile([128, 128], bf16, tag="sq")
                nc.tensor.transpose(tp[:], h2[:, 128 * k : 128 * (k + 1)], ident_sb[:])
                h2T = wpool.tile([128, 128], bf16, tag="h2T", bufs=3)
                nc.scalar.activation(h2T[:], tp[:], AF.Copy)
                nc.tensor.matmul(
                    h3_ps[:], lhsT=h2T[:], rhs=w2_sb[k][:], start=(k == 0), stop=(k == 2)
                )
            row = pack_row(t, h3_ps)
            r = rows_of(t)
            nc.sync.dma_start(out=agin2[TILE * t : TILE * t + r, :], in_=row[:r, :])

        nc.gpsimd.collective_compute(
            "AllGather",
            mybir.AluOpType.bypass,
            replica_groups=[list(range(NCORES))],
            ins=[agin2[:]],
            outs=[hext2[:]],
        )

        # ---------------- Phase 3: edge pass L2 + classifier ----------------
        for t in range(min(NTILES, limit)):
            out_ps = edge_pass(t, hext2)
            h3 = normalize(out_ps, b2r_sb)
            cls_ps = ppool.tile([128, N_CLASSES], f32, tag="dcol")
            for k in range(3):
                tp = ppool.tile([128, 128], bf16, tag="sq")
                nc.tensor.transpose(tp[:], h3[:, 128 * k : 128 * (k + 1)], ident_sb[:])
                h3T = wpool.tile([128, 128], bf16, tag="h2T", bufs=3)
                nc.scalar.activation(h3T[:], tp[:], AF.Copy)
                nc.tensor.matmul(
                    cls_ps[:], lhsT=h3T[:], rhs=outw_sb[k][:], start=(k == 0), stop=(k == 2)
                )
            outt = wpool.tile([128, N_CLASSES], f32, tag="outt")
            nc.vector.tensor_tensor(out=outt[:], in0=cls_ps[:], in1=outbr_sb[:], op=OP.add)
            r = rows_of(t)
            nc.sync.dma_start(out=OUT.ap()[TILE * t : TILE * t + r, :], in_=outt[:r, :])

    nc.compile()
    return nc


# =========================================================================
# entry point
# =========================================================================
def _prepare(inputs):
    """Build (cached) program + per-core input maps from FULL inputs."""
    import ml_dtypes

    x = np.asarray(inputs["x"], np.float32)
    edge_index = np.asarray(inputs["edge_index"])

    key = "prog"
    if key not in _CACHE:
        sched, IDX0, IDX1, ISEQ, ISEQT = _preprocess(edge_index)
        nc = _build_program(sched)
        _CACHE[key] = (sched, IDX0, IDX1, ISEQ, ISEQT, nc)
    sched, IDX0, IDX1, ISEQ, ISEQT, nc = _CACHE[key]

    shared = _shared_inputs(
        inputs["W1"], inputs["att_src1"], inputs["att_dst1"], inputs["b1"],
        inputs["W2"], inputs["att_src2"], inputs["att_dst2"], inputs["b2"],
        inputs["outW"], inputs["outb"],
    )

    in_maps = []
    for r in range(NCORES):
        xs = x[r * NPC : (r + 1) * NPC]  # [NPC, 256]
        xtt = np.zeros((NTILES * IN_DIM, TILE), ml_dtypes.bfloat16)
        for t in range(NTILES):
            rt = min(TILE, NPC - t * TILE)
            xtt[IN_DIM * t : IN_DIM * (t + 1), :rt] = (
                xs[TILE * t : TILE * t + rt].T.astype(ml_dtypes.bfloat16)
            )
        m = dict(shared)
        m["XTT"] = xtt
        m["IDX0"] = IDX0[r]
        m["IDX1"] = IDX1[r]
        m["ISEQ"] = ISEQ[r]
        m["ISEQT"] = ISEQT[r]
        in_maps.append(m)
    return nc, in_maps


def _assemble(results):
    return np.concatenate([results[r]["OUT"] for r in range(NCORES)], axis=0)


def kernel(**inputs):
    nc, in_maps = _prepare(inputs)

    from concourse.bass_utils import run_bass_kernel_spmd

    res = run_bass_kernel_spmd(nc, in_maps, core_ids=list(range(NCORES)))
    return _assemble(res.results)


if __name__ == "__main__":
    sys.path.insert(0, os.path.dirname(os.path.abspath(__file__)))
    import reference

    inp = {k: np.asarray(v) for k, v in reference.setup_inputs().items()}
    got = kernel(**inp)
    exp = np.asarray(reference.reference(**reference.setup_inputs()))
    err = np.abs(got - exp).max() / (np.abs(exp).max() + 1e-12)
    print("rel err:", err)


# revision 12
# speedup vs baseline: 1.9369x; 1.2932x over previous
"""GAT (2-layer + classifier) Trainium2 Bass kernel, 8-core SPMD.

Sharding: destination nodes (and their incoming edges, sorted by dst) are
sharded across 8 cores; projected node features are replicated via AllGather;
per-node softmax numerator and denominator are accumulated with the weighted
message matmul (ones-column trick), so no cross-core reduction is needed.

v5 design (on top of v4's fp8 rows / streamed static one-hots / W-augment):
- Each layer's AllGather is split in two (tiles 0..24 -> table A, 25..48 ->
  table B).  Source windows are (core, pos<3200) -> row of table A else B, so
  window-A gathers only depend on collective A and overlap collective B and
  the producer compute of the next layer.
- Self-loop edges live in a dedicated last chunk per tile filled by ONE
  contiguous static DMA from the core's own agin rows -- no gather
  descriptors (the Pool-engine descriptor generation is the critical path).
- relu on the scalar engine; the per-tile softmax scale is issued in two
  halves so the message matmuls start earlier.
"""

import os
import sys

import numpy as np

sys.path.insert(0, "/opt/trn_rl_repo")

# ---------------- problem constants (hardcoded, from the GAT spec) ---------
N_NODES = 50000
N_EDGES = 800000
IN_DIM = 256
HID = 128
HEADS = 3
N_CLASSES = 40
HC = HEADS * HID  # 384
NEG_SLOPE = 0.2

NCORES = 8
NPC = N_NODES // NCORES  # 6250 nodes per core
TILE = 128
NTILES = (NPC + TILE - 1) // TILE  # 49 (last tile has 106 rows)
TILA = 25  # tiles 0..24 -> table A
NPA = TILA * TILE  # 3200 rows/core in table A
NPB = NPC - NPA  # 3050 rows/core in table B
WINA = NCORES * NPA  # 25600
WINB = NCORES * NPB  # 24400
MMN = 387  # matmul N: 3*(128+1), interleaved [h|1] blocks
AUGC = 390  # augmented weight cols: h(384) + asrc(3) + adst(3)
FP8 = os.environ.get("GAT_FP8", "1") == "1"
ROWE = 512 if FP8 else 448  # gather row elems; bytes must be %256
ASRCB = 194 if FP8 else None  # bf16 idx of asrc within bitcast row (fp8)
ACT_HEAD = os.environ.get("GAT_ACT_HEAD", "1") == "1"  # head 2 scale on Act
RELU_ACT = os.environ.get("GAT_RELU_ACT", "1") == "1"
SPLIT_SCALE = os.environ.get("GAT_SPLIT_SCALE", "1") == "1"
SELF_CHUNK = os.environ.get("GAT_SELF_CHUNK", "1") == "1"

_CACHE = {}


def _round_up(x, m):
    return (x + m - 1) // m * m


def _bf16(a):
    import ml_dtypes

    return np.asarray(a, np.float32).astype(ml_dtypes.bfloat16)


# =========================================================================
# Host-side preprocessing: edge sort / shard / pad, idx + static one-hots
# =========================================================================
def _preprocess(edge_index):
    import ml_dtypes

    src = np.asarray(edge_index[0], dtype=np.int64)
    dst = np.asarray(edge_index[1], dtype=np.int64)
    # appended self loops go to the dedicated static chunk; natural (i,i)
    # edges (if any) stay in the normal gather lists

    core = dst // NPC
    rel = dst - core * NPC
    tile_i = rel // TILE
    spos = src % NPC
    win = (spos >= NPA).astype(np.int64)
    srow = np.where(win == 0, (src // NPC) * NPA + spos, (src // NPC) * NPB + spos - NPA)
    key = (core * NTILES + tile_i) * 2 + win
    order = np.argsort(key, kind="stable")
    srow = srow[order]
    rel = rel[order]

    counts = np.bincount(key[order], minlength=NCORES * NTILES * 2)
    starts = np.zeros(NCORES * NTILES * 2 + 1, np.int64)
    np.cumsum(counts, out=starts[1:])
    cnt = counts.reshape(NCORES, NTILES, 2)

    p0 = _round_up(cnt[:, :, 0].max(axis=0), 128)  # [NTILES] padded win0 len
    p1 = _round_up(cnt[:, :, 1].max(axis=0), 128)
    c_t = (p0 + p1) // 128 + 1  # chunks per tile (+1 self chunk)
    T0 = int(p0.sum())
    T1 = int(p1.sum())
    CTOT = int(c_t.sum())
    o0 = np.concatenate([[0], np.cumsum(p0)])  # idx elem offsets
    o1 = np.concatenate([[0], np.cumsum(p1)])
    oc = np.concatenate([[0], np.cumsum(c_t)])  # chunk offsets

    IDX0 = np.zeros((NCORES, 128, T0 // 16), np.int16)
    IDX1 = np.zeros((NCORES, 128, T1 // 16), np.int16)
    ISEQ = np.zeros((NCORES, 128, CTOT * 128), ml_dtypes.bfloat16)
    ISEQT = np.zeros((NCORES, 128, CTOT * 128), ml_dtypes.bfloat16)
    d128 = np.arange(128)

    def wrap16(a):  # idx i -> [i%16, i//16], replicated to 128 partitions
        w = a.reshape(-1, 16).T
        return np.tile(w, (8, 1))

    for r in range(NCORES):
        for t in range(NTILES):
            c = c_t[t]
            nedge = 128 * (c - 1)
            drel_tile = np.full(nedge, -1.0, np.float32)
            for w, (P, O, IDX, off_in) in enumerate(
                ((p0, o0, IDX0, 0), (p1, o1, IDX1, p0[t]))
            ):
                k = (r * NTILES + t) * 2 + w
                s, e = starts[k], starts[k + 1]
                n = e - s
                idxs = np.zeros(P[t], np.int16)
                if n > 0:
                    idxs[:n] = srow[s:e].astype(np.int16)
                    idxs[n:] = idxs[n - 1]
                    drel_tile[off_in : off_in + n] = (rel[s:e] - t * TILE).astype(
                        np.float32
                    )
                if P[t] > 0:
                    IDX[r, :, O[t] // 16 : (O[t] + P[t]) // 16] = wrap16(idxs)
            dt = drel_tile.reshape(c - 1, 128)  # [chunk, edge-slot]
            oh = dt[:, :, None] == d128[None, None, :]  # [c-1, e, d]
            rt = min(TILE, NPC - t * TILE)
            eye = (d128[:, None] == d128[None, :]) & (d128[:, None] < rt)
            oh = np.concatenate([oh, eye[None]], axis=0)  # [c, e, d]
            sl = slice(oc[t] * 128, (oc[t] + c) * 128)
            ISEQ[r, :, sl] = (
                oh.transpose(1, 0, 2).reshape(128, c * 128).astype(ml_dtypes.bfloat16)
            )
            ISEQT[r, :, sl] = (
                oh.transpose(2, 0, 1).reshape(128, c * 128).astype(ml_dtypes.bfloat16)
            )

    sched = dict(
        p0=[int(v) for v in p0],
        p1=[int(v) for v in p1],
        c_t=[int(v) for v in c_t],
        o0=[int(v) for v in o0],
        o1=[int(v) for v in o1],
        oc=[int(v) for v in oc],
        T0=T0,
        T1=T1,
        CTOT=CTOT,
    )
    return sched, IDX0, IDX1, ISEQ, ISEQT


def _augment(W, att_src, att_dst):
    """[K, 384] -> [K, 390] with per-head att_src/att_dst projections."""
    W = np.asarray(W, np.float32)
    cols = [W]
    for att in (att_src, att_dst):
        a = np.zeros((W.shape[0], HEADS), np.float32)
        for h in range(HEADS):
            a[:, h] = W[:, HID * h : HID * (h + 1)] @ np.asarray(att[h], np.float32)
        cols.append(a)
    return np.concatenate(cols, axis=1)  # [K, 390]


def _shared_inputs(W1, att_src1, att_dst1, b1, W2, att_src2, att_dst2, b2, outW, outb):
    f = np.float32
    return {
        "W1A": _bf16(_augment(W1, att_src1, att_dst1)),  # [256, 390]
        "W2A": _bf16(_augment(W2, att_src2, att_dst2)),  # [384, 390]
        "OUTW": _bf16(outW),  # [384, 40]
        "B1R": _bf16(np.tile(np.asarray(b1, f)[None, :], (128, 1))),
        "B2R": _bf16(np.tile(np.asarray(b2, f)[None, :], (128, 1))),
        "OUTBR": np.tile(np.asarray(outb, f)[None, :], (128, 1)),
        "IDENT": _bf16(np.eye(128, dtype=f)),
    }


# =========================================================================
# Bass program
# =========================================================================
def _build_program(sched):
    from contextlib import ExitStack

    import concourse.bass as bass
    import concourse.mybir as mybir
    import concourse.tile as tile
    from concourse import bacc

    f32 = mybir.dt.float32
    bf16 = mybir.dt.bfloat16
    fp8 = mybir.dt.float8e4
    i16 = mybir.dt.int16
    gdt = fp8 if FP8 else f32
    AF = mybir.ActivationFunctionType
    OP = mybir.AluOpType
    AP = bass.AP

    p0, p1, c_t = sched["p0"], sched["p1"], sched["c_t"]
    o0, o1, oc = sched["o0"], sched["o1"], sched["oc"]
    T0, T1, CTOT = sched["T0"], sched["T1"], sched["CTOT"]

    nc = bacc.Bacc(
        "TRN2",
        target_bir_lowering=False,
        debug=False,
        enable_asserts=False,
        num_devices=NCORES,
        num_swdge_queues=2,
        dynamic_dma_scratch_size=int(os.environ.get("GAT_DMA_SCRATCH", 16384)),
    )

    # ---- I/O ----
    XTT = nc.dram_tensor("XTT", [NTILES * IN_DIM, TILE], bf16, kind="ExternalInput")
    IDX0 = nc.dram_tensor("IDX0", [128, T0 // 16], i16, kind="ExternalInput")
    IDX1 = nc.dram_tensor("IDX1", [128, T1 // 16], i16, kind="ExternalInput")
    ISEQ = nc.dram_tensor("ISEQ", [128, CTOT * 128], bf16, kind="ExternalInput")
    ISEQT = nc.dram_tensor("ISEQT", [128, CTOT * 128], bf16, kind="ExternalInput")
    W1A = nc.dram_tensor("W1A", [IN_DIM, AUGC], bf16, kind="ExternalInput")
    W2A = nc.dram_tensor("W2A", [HC, AUGC], bf16, kind="ExternalInput")
    OUTW = nc.dram_tensor("OUTW", [HC, N_CLASSES], bf16, kind="ExternalInput")
    B1R = nc.dram_tensor("B1R", [128, HC], bf16, kind="ExternalInput")
    B2R = nc.dram_tensor("B2R", [128, HC], bf16, kind="ExternalInput")
    OUTBR = nc.dram_tensor("OUTBR", [128, N_CLASSES], f32, kind="ExternalInput")
    IDENT = nc.dram_tensor("IDENT", [128, 128], bf16, kind="ExternalInput")
    OUT = nc.dram_tensor("OUT", [NPC, N_CLASSES], f32, kind="ExternalOutput")

    def strided3(ap2d, start, step, count):
        # [128, N] -> [128, count] picking cols start, start+step, ...
        base = ap2d[:, start : start + 1]
        return AP(base.tensor, base.offset, [base.ap[0], [step, count]])

    def seg_view(ap2d, nseg, seglen, stride):
        # [128, N] -> [128, nseg, seglen] with segment stride `stride`
        return AP(ap2d.tensor, ap2d.offset, [ap2d.ap[0], [stride, nseg], [1, seglen]])

    with tile.TileContext(nc) as tc, ExitStack() as ctx:
        cpool = ctx.enter_context(tc.tile_pool(name="cpool", bufs=1))
        dram = ctx.enter_context(tc.tile_pool(name="dram", bufs=1, space="DRAM"))
        gpool = ctx.enter_context(tc.tile_pool(name="gpool", bufs=2))
        wpool = ctx.enter_context(tc.tile_pool(name="wpool", bufs=2))
        ppool = ctx.enter_context(tc.tile_pool(name="ppool", bufs=2, space="PSUM"))
        apool = ctx.enter_context(tc.tile_pool(name="apool", bufs=3, space="PSUM"))

        # resident constants
        idx0_sb = cpool.tile_from(IDX0.ap())
        idx1_sb = cpool.tile_from(IDX1.ap())
        b1r_sb = cpool.tile_from(B1R.ap())
        b2r_sb = cpool.tile_from(B2R.ap())
        outbr_sb = cpool.tile_from(OUTBR.ap())
        ident_sb = cpool.tile_from(IDENT.ap())
        w1_sb = [
            cpool.tile_from(W1A.ap()[128 * k : 128 * (k + 1), :], name=f"w1_{k}")
            for k in range(2)
        ]
        w2_sb = [
            cpool.tile_from(W2A.ap()[128 * k : 128 * (k + 1), :], name=f"w2_{k}")
            for k in range(3)
        ]
        outw_sb = [
            cpool.tile_from(OUTW.ap()[128 * k : 128 * (k + 1), :], name=f"outw_{k}")
            for k in range(3)
        ]
        adst = cpool.tile([128, NTILES * HEADS], bf16)  # per-layer a_dst per tile

        aginA1 = dram.tile([NPA, ROWE], gdt)
        aginB1 = dram.tile([NPB, ROWE], gdt)
        hextA1 = dram.tile([WINA, ROWE], gdt, addr_space="Shared")
        hextB1 = dram.tile([WINB, ROWE], gdt, addr_space="Shared")
        aginA2 = dram.tile([NPA, ROWE], gdt)
        aginB2 = dram.tile([NPB, ROWE], gdt)
        hextA2 = dram.tile([WINA, ROWE], gdt, addr_space="Shared")
        hextB2 = dram.tile([WINB, ROWE], gdt, addr_space="Shared")

        def rows_of(t):
            return min(TILE, NPC - t * TILE)

        def agin_slice(aginA, aginB, t):
            r = rows_of(t)
            if t < TILA:
                return aginA[TILE * t : TILE * t + r, :]
            return aginB[TILE * (t - TILA) : TILE * (t - TILA) + r, :]

        def pack_row(t, src_psum):
            """psum [128, 390] = [h(384)|asrc(3)|adst(3)] -> table row tile."""
            row = wpool.tile([128, ROWE], gdt, tag="row")
            nc.gpsimd.memset(row[:, MMN:ROWE], 0.0)
            nc.vector.tensor_copy(
                seg_view(row, HEADS, HID, HID + 1), seg_view(src_psum, HEADS, HID, HID)
            )
            nc.vector.memset(strided3(row, HID, HID + 1, HEADS), 1.0)
            if FP8:
                rb = row[:].bitcast(bf16)  # [128, 256]
                nc.vector.tensor_copy(
                    rb[:, ASRCB : ASRCB + 3], src_psum[:, HC : HC + 3]
                )
            else:
                nc.vector.tensor_copy(row[:, MMN : MMN + 3], src_psum[:, HC : HC + 3])
            nc.vector.tensor_copy(
                adst[:, HEADS * t : HEADS * (t + 1)], src_psum[:, HC + 3 : HC + 6]
            )
            return row

        def phase1_tile(t):
            h1_ps = apool.tile([128, AUGC], f32, tag="acc")
            for k in range(2):
                xk = wpool.tile([128, 128], bf16, tag="xk")
                nc.sync.dma_start(
                    out=xk[:],
                    in_=XTT.ap()[IN_DIM * t + 128 * k : IN_DIM * t + 128 * (k + 1), :],
                )
                nc.tensor.matmul(
                    h1_ps[:], lhsT=xk[:], rhs=w1_sb[k][:], start=(k == 0), stop=(k == 1)
                )
            row = pack_row(t, h1_ps)
            nc.sync.dma_start(out=agin_slice(aginA1, aginB1, t), in_=row[: rows_of(t), :])

        def allgather(agin, hext):
            nc.gpsimd.collective_compute(
                "AllGather",
                mybir.AluOpType.bypass,
                replica_groups=[list(range(NCORES))],
                ins=[agin[:]],
                outs=[hext[:]],
            )

        # ---------------- Phase 1: h1 = x @ W1A, pack, A/B AllGather --------
        for t in range(TILA):
            phase1_tile(t)
        allgather(aginA1, hextA1)
        for t in range(TILA, NTILES):
            phase1_tile(t)
        allgather(aginB1, hextB1)

        # ---------------- Edge pass (shared for both layers) ----------------
        def edge_pass(t, hextA, hextB, aginA, aginB):
            c = c_t[t]
            q0 = p0[t] // 128
            q01 = q0 + p1[t] // 128
            G = gpool.tile([128, c, ROWE], gdt, tag="G")
            if p0[t] > 0:
                nc.gpsimd.dma_gather(
                    out_ap=G[:, :q0, :],
                    in_ap=hextA[0:WINA, :],
                    idxs_ap=idx0_sb[:, o0[t] // 16 : (o0[t] + p0[t]) // 16],
                    num_idxs=p0[t],
                    num_idxs_reg=p0[t],
                    elem_size=ROWE,
                    queue_num=0,
                    single_packet=False,
                )
            if p1[t] > 0:
                nc.gpsimd.dma_gather(
                    out_ap=G[:, q0:q01, :],
                    in_ap=hextB[0:WINB, :],
                    idxs_ap=idx1_sb[:, o1[t] // 16 : (o1[t] + p1[t]) // 16],
                    num_idxs=p1[t],
                    num_idxs_reg=p1[t],
                    elem_size=ROWE,
                    queue_num=1,
                    single_packet=False,
                )
            # self-loop chunk: contiguous read of this core's own packed rows
            r = rows_of(t)
            if r < TILE:
                nc.vector.memset(G[:, c - 1, :], 0.0)
            nc.sync.dma_start(
                out=G[:r, c - 1, :], in_=agin_slice(aginA, aginB, t)
            )
            # stream the static one-hot blocks for this tile
            iseq = wpool.tile([128, c, 128], bf16, tag="iseq")
            nc.sync.dma_start(
                out=iseq[:], in_=ISEQ.ap()[:, oc[t] * 128 : (oc[t] + c) * 128]
            )
            iseqT = wpool.tile([128, c, 128], bf16, tag="iseqT")
            nc.sync.dma_start(
                out=iseqT[:], in_=ISEQT.ap()[:, oc[t] * 128 : (oc[t] + c) * 128]
            )
            # a_dst per edge: dcol[:, ci, :] = iseqT_ci^T @ adst_t
            dcol_ps = ppool.tile([128, c, HEADS], f32, tag="dcol")
            for ci in range(c):
                nc.tensor.matmul(
                    dcol_ps[:, ci, :],
                    lhsT=iseqT[:, ci, :],
                    rhs=adst[:, HEADS * t : HEADS * (t + 1)],
                    start=True,
                    stop=True,
                )
            # alpha / leaky relu / exp   [128, c, 3]
            if FP8:
                gb = G[:].bitcast(bf16)  # [128, c, 256]
                asrcv = gb[:, :, ASRCB : ASRCB + 3]
            else:
                asrcv = G[:, :, MMN : MMN + 3]
            alpha = wpool.tile([128, c, HEADS], bf16, tag="alpha")
            nc.vector.tensor_tensor(
                out=alpha[:], in0=asrcv, in1=dcol_ps[:], op=OP.add
            )
            nc.vector.scalar_tensor_tensor(
                out=alpha[:], in0=alpha[:], scalar=NEG_SLOPE, in1=alpha[:],
                op0=OP.mult, op1=OP.max,
            )
            ex = wpool.tile([128, c, HEADS], f32, tag="ex")
            nc.scalar.activation(ex[:], alpha[:], AF.Exp)
            # Gs = G[0:387] * ex (per 129-col head block; ones cols -> denom)
            Gs = gpool.tile([128, c, MMN], bf16, tag="Gs")
            nh = 2 if ACT_HEAD else 3
            ch = c // 2 if SPLIT_SCALE else 0
            for a, b in (((0, ch), (ch, c)) if SPLIT_SCALE else ((0, c),)):
                n = b - a
                gt, st, et = G[:], Gs[:], ex[:]
                g4 = AP(gt.tensor, gt.offset + a * ROWE,
                        [gt.ap[0], [ROWE, n], [HID + 1, nh], [1, HID + 1]])
                s4 = AP(st.tensor, st.offset + a * MMN,
                        [st.ap[0], [MMN, n], [HID + 1, nh], [1, HID + 1]])
                e4 = AP(et.tensor, et.offset + a * HEADS,
                        [et.ap[0], [HEADS, n], [1, nh], [0, HID + 1]])
                nc.vector.tensor_tensor(out=s4, in0=g4, in1=e4, op=OP.mult)
            if ACT_HEAD:
                for ci in range(c):
                    nc.scalar.activation(
                        Gs[:, ci, 2 * (HID + 1) : MMN],
                        G[:, ci, 2 * (HID + 1) : MMN],
                        AF.Copy,
                        scale=ex[:, ci, 2:3],
                    )
            # weighted segment sum (and denominators via the ones columns)
            out_ps = apool.tile([128, MMN], f32, tag="acc")
            for ci in range(c):
                nc.tensor.matmul(
                    out_ps[:],
                    lhsT=iseq[:, ci, :],
                    rhs=Gs[:, ci, :],
                    start=(ci == 0),
                    stop=(ci == c - 1),
                )
            return out_ps

        def normalize(out_ps, brep_sb):
            """h = relu(out/denom + bias)  -> [128, 384] bf16 sbuf tile"""
            tmp3 = wpool.tile([128, HEADS], f32, tag="tmp3")
            nc.vector.tensor_scalar_add(
                tmp3[:], strided3(out_ps, HID, HID + 1, HEADS), 1e-16
            )
            r3 = wpool.tile([128, HEADS], f32, tag="r3")
            nc.vector.reciprocal(r3[:], tmp3[:])
            h2 = wpool.tile([128, HC], bf16, tag="h2")
            for h in range(HEADS):
                nc.vector.scalar_tensor_tensor(
                    out=h2[:, HID * h : HID * (h + 1)],
                    in0=out_ps[:, (HID + 1) * h : (HID + 1) * h + HID],
                    scalar=r3[:, h : h + 1],
                    in1=brep_sb[:, HID * h : HID * (h + 1)],
                    op0=OP.mult,
                    op1=OP.add,
                )
            if RELU_ACT:
                nc.scalar.activation(h2[:], h2[:], AF.Relu)
            else:
                nc.vector.tensor_scalar_max(h2[:], h2[:], 0.0)
            return h2

        # ---------------- Phase 2: edge pass L1 + entry L2 ------------------
        limit = int(os.environ.get("GAT_LIMIT_TILES", NTILES))

        def phase2_tile(t):
            out_ps = edge_pass(t, hextA1, hextB1, aginA1, aginB1)
            h2 = normalize(out_ps, b1r_sb)
            h3_ps = apool.tile([128, AUGC], f32, tag="acc")
            for k in range(3):
                tp = ppool.tile([128, 128], bf16, tag="sq")
                nc.tensor.transpose(tp[:], h2[:, 128 * k : 128 * (k + 1)], ident_sb[:])
                h2T = wpool.tile([128, 128], bf16, tag="h2T", bufs=3)
                nc.scalar.activation(h2T[:], tp[:], AF.Copy)
                nc.tensor.matmul(
                    h3_ps[:], lhsT=h2T[:], rhs=w2_sb[k][:], start=(k == 0), stop=(k == 2)
                )
            row = pack_row(t, h3_ps)
            nc.sync.dma_start(out=agin_slice(aginA2, aginB2, t), in_=row[: rows_of(t), :])

        for t in range(min(TILA, limit)):
            phase2_tile(t)
        allgather(aginA2, hextA2)
        for t in range(TILA, min(NTILES, limit)):
            phase2_tile(t)
        allgather(aginB2, hextB2)

        # ---------------- Phase 3: edge pass L2 + classifier ----------------
        for t in range(min(NTILES, limit)):
            out_ps = edge_pass(t, hextA2, hextB2, aginA2, aginB2)
            h3 = normalize(out_ps, b2r_sb)
            cls_ps = ppool.tile([128, N_CLASSES], f32, tag="dcol")
            for k in range(3):
                tp = ppool.tile([128, 128], bf16, tag="sq")
                nc.tensor.transpose(tp[:], h3[:, 128 * k : 128 * (k + 1)], ident_sb[:])
                h3T = wpool.tile([128, 128], bf16, tag="h2T", bufs=3)
                nc.scalar.activation(h3T[:], tp[:], AF.Copy)
                nc.tensor.matmul(
                    cls_ps[:], lhsT=h3T[:], rhs=outw_sb[k][:], start=(k == 0), stop=(k == 2)
                )
            outt = wpool.tile([128, N_CLASSES], f32, tag="outt")
            nc.vector.tensor_tensor(out=outt[:], in0=cls_ps[:], in1=outbr_sb[:], op=OP.add)
            r = rows_of(t)
            nc.sync.dma_start(out=OUT.ap()[TILE * t : TILE * t + r, :], in_=outt[:r, :])

    nc.compile()
    return nc


# =========================================================================
# entry point
# =========================================================================
def _prepare(inputs):
    """Build (cached) program + per-core input maps from FULL inputs."""
    import ml_dtypes

    x = np.asarray(inputs["x"], np.float32)
    edge_index = np.asarray(inputs["edge_index"])

    key = "prog"
    if key not in _CACHE:
        sched, IDX0, IDX1, ISEQ, ISEQT = _preprocess(edge_index)
        nc = _build_program(sched)
        _CACHE[key] = (sched, IDX0, IDX1, ISEQ, ISEQT, nc)
    sched, IDX0, IDX1, ISEQ, ISEQT, nc = _CACHE[key]

    shared = _shared_inputs(
        inputs["W1"], inputs["att_src1"], inputs["att_dst1"], inputs["b1"],
        inputs["W2"], inputs["att_src2"], inputs["att_dst2"], inputs["b2"],
        inputs["outW"], inputs["outb"],
    )

    in_maps = []
    for r in range(NCORES):
        xs = x[r * NPC : (r + 1) * NPC]  # [NPC, 256]
        xtt = np.zeros((NTILES * IN_DIM, TILE), ml_dtypes.bfloat16)
        for t in range(NTILES):
            rt = min(TILE, NPC - t * TILE)
            xtt[IN_DIM * t : IN_DIM * (t + 1), :rt] = (
                xs[TILE * t : TILE * t + rt].T.astype(ml_dtypes.bfloat16)
            )
        m = dict(shared)
        m["XTT"] = xtt
        m["IDX0"] = IDX0[r]
        m["IDX1"] = IDX1[r]
        m["ISEQ"] = ISEQ[r]
        m["ISEQT"] = ISEQT[r]
        in_maps.append(m)
    return nc, in_maps


def _assemble(results):
    return np.concatenate([results[r]["OUT"] for r in range(NCORES)], axis=0)


def kernel(**inputs):
    nc, in_maps = _prepare(inputs)

    from concourse.bass_utils import run_bass_kernel_spmd

    res = run_bass_kernel_spmd(nc, in_maps, core_ids=list(range(NCORES)))
    return _assemble(res.results)


if __name__ == "__main__":
    sys.path.insert(0, os.path.dirname(os.path.abspath(__file__)))
    import reference

    inp = {k: np.asarray(v) for k, v in reference.setup_inputs().items()}
    got = kernel(**inp)
    exp = np.asarray(reference.reference(**reference.setup_inputs()))
    err = np.abs(got - exp).max() / (np.abs(exp).max() + 1e-12)
    print("rel err:", err)
